# revision 1
# baseline (speedup 1.0000x reference)
"""Trainium2 Bass kernel for nn_Classifier_6717328851414.

DEQ-style classifier:
  150 iterations of  z <- 0.5*z + 0.5*lrelu(conv2(lrelu(conv1(cat(z, img)))))
  with conv1: 8->6 ch, 5x5 pad 2; conv2: 6->5 ch, 5x5 pad 2; 32x32 images,
  then a 5->10 channel 32x32 "head" conv (valid) producing logits (N,10,1,1).

Strategy: pure data parallel over batch N=512 -> 64 images per NeuronCore.

Per-core layout (all SBUF-resident, fp32):
  partitions = (channel_local, x)  i.e. p = c*32 + x
  free       = (y_padded, n)       i.e. f = y*64 + n, y in [0,36) with rows
                                   0,1,34,35 zero (conv pad=2), data y+2.
  hA  [128, 2304]: z channels 0..3          (conv1 input chunk A / conv2 out)
  hB  [128, 2304]: rows 0:32 z ch4, rows 32:128 image ch 0..2 (static)
  h1A [96, 2304]:  hidden channels 0..2
  h1B [96, 2304]:  hidden channels 3..5

Convs are computed as banded matmuls on the TensorEngine: for each kernel
row ky (5), contraction chunk (2) and output chunk, a matmul with stationary
B[(ci,x),(co,x')] = w[co,ci,ky,x-x'+2] accumulates into PSUM; the y shift of
ky is applied by offsetting the moving AP by ky rows in the padded slab.
Output y-quarters of 8 rows x 64 images give contiguous 512-elem moving APs.
Matmuls run in fp32r (full PE rate at free dim 512, fp32 storage).

Weights/biases are pre-transformed into these banded stationary layouts on
the host (numpy) inside kernel().
"""

import numpy as np

import concourse.bass as bass
import concourse.mybir as mybir
import concourse.tile as tile
from concourse.vector_clock import ScopedClock, VectorClock

ITERS = 150
SLOPE = 0.01
ALPHA = 0.5
NCORES = 8
NTOT = 512
NPER = NTOT // NCORES  # 64
Y = 36  # padded y
FREE = Y * NPER  # 2304
F32 = mybir.dt.float32
F32R = mybir.dt.float32r
F16 = mybir.dt.float16
AF = mybir.ActivationFunctionType
OP = mybir.AluOpType


def _patched_drain_and_barrier(self, tick_clock, wait_clock):
    # Workaround: this walrus rejects >2 sync waits on one instruction
    # ("Too many sync wait commands"). Split the final drain's waits across
    # one SP nop per logical processor.
    gc = tick_clock.global_clock
    n = len(gc)
    for p in range(n):
        if gc[p] == 0:
            continue
        vc = VectorClock([gc[q] if q == p else 0 for q in range(n)])
        nop = self.nc.sync.nop(nofuse=True)
        wait_clock.add_sem_waits(nop.ins, ScopedClock({None: vc}))
    self.nc.sync.drain()
    self.nc.all_engine_barrier()
    assert self.sems is not None
    popped = self.nc._tile_sem_poison_stack.pop()
    assert popped is self._sem_poison
    self.nc.clear_and_free_semaphores(list(self.sems.allocated().values()))
    self.nc.all_engine_barrier()


tile.TileContext._drain_and_barrier = _patched_drain_and_barrier


def _split_excess_waits(nc, limit=1):
    """Walrus codegen rejects instructions with >2 sync waits (>1 for the
    self-loading fp32 matmul's LDWEIGHTS struct); hoist the excess onto
    same-engine NoOps placed immediately before."""
    for bb in nc.main_func.blocks:
        out = []
        changed = False
        for ins in bb.instructions:
            lim = limit
            si = ins.sync_info
            waits = list(si.on_wait) if (si is not None and si.on_wait) else []
            if len(waits) > lim:
                extra, keep = waits[:-lim], waits[-lim:]
                for i0 in range(0, len(extra), limit):
                    nop = mybir.InstNoOp(
                        name=nc.get_next_instruction_name(),
                        engine=ins.engine,
                        ins=[],
                        outs=[],
                        sync_info=mybir.SyncInfo(
                            on_wait=extra[i0 : i0 + limit], on_update=[]
                        ),
                    )
                    out.append(nop)
                si.on_wait = keep
                changed = True
            out.append(ins)
        if changed:
            bb.instructions = out


def _c1col(ky, cc, oc):
    return ((ky * 2 + cc) * 2 + oc) * 96


def _c2col(ky, cc):
    return (ky * 2 + cc) * 160


def build_nc(iters=ITERS, unroll=5):
    nc = bass.Bass()

    img_p = nc.declare_dram_parameter("img", [96, 4 * 12 * NPER], F16, isOutput=False)
    w1s_p = nc.declare_dram_parameter("w1s", [128, 1920], F16, isOutput=False)
    w2s_p = nc.declare_dram_parameter("w2s", [96, 1600], F16, isOutput=False)
    whsa_p = nc.declare_dram_parameter("whsa", [128, 320], F32R, isOutput=False)
    whsb_p = nc.declare_dram_parameter("whsb", [32, 320], F32R, isOutput=False)
    bias_p = nc.declare_dram_parameter("bias", [128, 8], F32, isOutput=False)
    out_p = nc.declare_dram_parameter("out", [10, NPER], F32, isOutput=True)

    with tile.TileContext(nc) as tc:
        with (
            tc.tile_pool(name="const", bufs=1) as cpool,
            tc.tile_pool(name="state", bufs=1) as spool,
            tc.tile_pool(name="psum", bufs=8, space="PSUM") as ppool,
            tc.tile_pool(name="stage", bufs=4) as vpool,
        ):
            w1s = cpool.tile([128, 1920], F16, tag="w1s")
            w2s = cpool.tile([96, 1600], F16, tag="w2s")
            whsa = cpool.tile([128, 320], F32R, tag="whsa")
            whsb = cpool.tile([32, 320], F32R, tag="whsb")
            bias = cpool.tile([128, 8], F32, tag="bias")
            nc.sync.dma_start(w1s[:], w1s_p[:])
            nc.sync.dma_start(w2s[:], w2s_p[:])
            nc.sync.dma_start(whsa[:], whsa_p[:])
            nc.sync.dma_start(whsb[:], whsb_p[:])
            nc.sync.dma_start(bias[:], bias_p[:])

            # Quarter-split slabs: tile q holds global y-rows 8q..8q+11
            # (12 rows x 64 images); rows 0..1 / 10..11 are halos duplicated
            # from neighbours so each quarter's conv reads stay in one tile.
            QF = 12 * NPER
            CEN = 2 * NPER
            hA = [spool.tile([128, QF], F32, tag=f"hA{q}", name=f"hA{q}") for q in range(4)]
            hB = [spool.tile([32, QF], F32, tag=f"hB{q}", name=f"hB{q}") for q in range(4)]
            hAs = [spool.tile([128, QF], F16, tag=f"hAs{q}", name=f"hAs{q}") for q in range(4)]
            hBs = [spool.tile([128, QF], F16, tag=f"hBs{q}", name=f"hBs{q}") for q in range(4)]
            h1A = [spool.tile([96, QF], F16, tag=f"h1A{q}", name=f"h1A{q}") for q in range(4)]
            h1B = [spool.tile([96, QF], F16, tag=f"h1B{q}", name=f"h1B{q}") for q in range(4)]
            for q in range(4):
                nc.gpsimd.memset(hA[q][:], 0.0)
                nc.gpsimd.memset(hB[q][:, :], 0.0)
                nc.gpsimd.memset(hAs[q][:], 0.0)
                nc.gpsimd.memset(hBs[q][:, :], 0.0)
                nc.gpsimd.memset(h1A[q][:], 0.0)
                nc.gpsimd.memset(h1B[q][:], 0.0)
                nc.sync.dma_start(
                    hBs[q][32:128, :], img_p[:, q * QF : (q + 1) * QF]
                )

            def jrange(q, ky):
                # output rows j with non-pad input rows (global row in 2..33)
                r0 = 8 * q + ky
                return max(0, 2 - r0), min(8, 34 - r0)

            def one_iter():
                # ---- conv1: h(8ch) -> h1(6ch)
                ps1 = {}
                for q in range(4):
                    for oc in range(2):
                        ps = ppool.tile([96, 512], F32, tag="ps")
                        ps1[(q, oc)] = ps
                        k = 0
                        for ky in range(5):
                            jlo, jhi = jrange(q, ky)
                            for cc, slabs in ((0, hAs), (1, hBs)):
                                c1 = _c1col(ky, cc, oc)
                                nc.tensor.matmul(
                                    ps[:, jlo * NPER : jhi * NPER],
                                    w1s[:, c1 : c1 + 96],
                                    slabs[q][:, (ky + jlo) * NPER : (ky + jhi) * NPER],
                                    start=(k == 0),
                                    stop=(k == 9),
                                )
                                k += 1
                for q in range(4):
                    for oc, h1s in ((0, h1A), (1, h1B)):
                        ps = ps1[(q, oc)]
                        t = h1s[q]
                        dst = t[:, CEN : CEN + 512]
                        nc.scalar.activation(dst, ps[:], AF.Identity, bias=bias[0:96, oc : oc + 1], scale=1.0)
                        nc.vector.scalar_tensor_tensor(dst, dst, SLOPE, dst, OP.mult, OP.max)
                        if q > 0:
                            nc.vector.tensor_copy(h1s[q - 1][:, 10 * NPER : 12 * NPER], t[:, 2 * NPER : 4 * NPER])
                        if q < 3:
                            nc.vector.tensor_copy(h1s[q + 1][:, 0 : 2 * NPER], t[:, 8 * NPER : 10 * NPER])

                # ---- conv2: h1(6ch) -> z update (5ch)
                ps2 = {}
                for q in range(4):
                    for oc, osz in ((0, 128), (1, 32)):
                        ps = ppool.tile([osz, 512], F32, tag="ps")
                        ps2[(q, oc)] = ps
                        k = 0
                        for ky in range(5):
                            jlo, jhi = jrange(q, ky)
                            for cc, h1s in ((0, h1A), (1, h1B)):
                                c0 = _c2col(ky, cc) + (0 if oc == 0 else 128)
                                nc.tensor.matmul(
                                    ps[:, jlo * NPER : jhi * NPER],
                                    w2s[:, c0 : c0 + osz],
                                    h1s[q][:, (ky + jlo) * NPER : (ky + jhi) * NPER],
                                    start=(k == 0),
                                    stop=(k == 9),
                                )
                                k += 1
                for q in range(4):
                    for oc, osz, zs in ((0, 128, hA), (1, 32, hB)):
                        ps = ps2[(q, oc)]
                        v = vpool.tile([osz, 512], F32, tag="v")
                        nc.scalar.activation(
                            v[:], ps[:], AF.Identity, bias=bias[0:osz, (2 + oc) : (3 + oc)], scale=0.5
                        )
                        nc.vector.scalar_tensor_tensor(v[:], v[:], SLOPE, v[:], OP.mult, OP.max)
                        t = zs[q]
                        dst = t[0:osz, CEN : CEN + 512]
                        nc.vector.scalar_tensor_tensor(dst.bitcast(F32R), dst, 0.5, v[:], OP.mult, OP.add)
                        if q > 0:
                            nc.vector.tensor_copy(zs[q - 1][0:osz, 10 * NPER : 12 * NPER].bitcast(F32R), t[0:osz, 2 * NPER : 4 * NPER])
                        if q < 3:
                            nc.vector.tensor_copy(zs[q + 1][0:osz, 0 : 2 * NPER].bitcast(F32R), t[0:osz, 8 * NPER : 10 * NPER])
                # refresh fp16 z shadows (full 12-row window incl halos)
                for q in range(4):
                    nc.vector.tensor_copy(hAs[q][:, :], hA[q][:, :])
                    nc.vector.tensor_copy(hBs[q][0:32, :], hB[q][0:32, :])

            trips, rem = divmod(iters, unroll)
            if trips > 0:
                with tc.For_i(0, trips, 1):
                    for _ in range(unroll):
                        one_iter()
            for _ in range(rem):
                one_iter()

            # ---- head: logits[k, n] = sum_{c,y,x} wh * z + bh
            psh = ppool.tile([10, NPER], F32, tag="ps")
            k = 0
            for y in range(32):
                q, r = divmod(y, 8)
                off = (r + 2) * NPER
                nc.tensor.matmul(
                    psh[:],
                    whsa[:, y * 10 : (y + 1) * 10].bitcast(F32R),
                    hA[q][:, off : off + NPER].bitcast(F32R),
                    start=(k == 0),
                    stop=False,
                )
                k += 1
                nc.tensor.matmul(
                    psh[:],
                    whsb[:, y * 10 : (y + 1) * 10].bitcast(F32R),
                    hB[q][0:32, off : off + NPER].bitcast(F32R),
                    start=False,
                    stop=(y == 31),
                )
                k += 1
            out_sb = vpool.tile([10, NPER], F32, tag="osb")
            nc.scalar.activation(out_sb[:], psh[:], AF.Identity, bias=bias[0:10, 4:5], scale=1.0)
            nc.sync.dma_start(out_p[:], out_sb[:])

    _split_excess_waits(nc)
    return nc


def pack_inputs(image, w1, b1, w2, b2, wh, bh):
    """Host-side transforms; returns (shared dict, per-core img slabs list)."""
    image = np.asarray(image, dtype=np.float32)
    w1 = np.asarray(w1, dtype=np.float32)
    b1 = np.asarray(b1, dtype=np.float32)
    w2 = np.asarray(w2, dtype=np.float32)
    b2 = np.asarray(b2, dtype=np.float32)
    wh = np.asarray(wh, dtype=np.float32)
    bh = np.asarray(bh, dtype=np.float32)

    # conv1 banded stationaries: [128, 1920]
    w1s = np.zeros((5, 2, 2, 128, 96), np.float32)
    for ky in range(5):
        for cc in range(2):
            for oc in range(2):
                for cis in range(4):
                    ci = cc * 4 + cis
                    for cos in range(3):
                        co = oc * 3 + cos
                        for dx in range(-2, 3):  # kx = dx + 2, x = x' + dx
                            kx = dx + 2
                            xs = np.arange(32)
                            xps = xs - dx
                            m = (xps >= 0) & (xps < 32)
                            w1s[ky, cc, oc, cis * 32 + xs[m], cos * 32 + xps[m]] = w1[co, ci, ky, kx]
    w1s = w1s.transpose(3, 0, 1, 2, 4).reshape(128, 1920)

    # conv2 banded stationaries: [96, 1600]; block (ky, cc): cols 0:128 z ch0..3, 128:160 z ch4
    w2s = np.zeros((5, 2, 96, 160), np.float32)
    for ky in range(5):
        for cc in range(2):
            for cis in range(3):
                ci = cc * 3 + cis
                for co in range(5):
                    base = co * 32 if co < 4 else 128
                    for dx in range(-2, 3):
                        kx = dx + 2
                        xs = np.arange(32)
                        xps = xs - dx
                        m = (xps >= 0) & (xps < 32)
                        w2s[ky, cc, cis * 32 + xs[m], base + xps[m]] = w2[co, ci, ky, kx]
    w2s = w2s.transpose(2, 0, 1, 3).reshape(96, 1600)

    # head stationaries
    whsa = np.zeros((128, 32, 10), np.float32)
    whsb = np.zeros((32, 32, 10), np.float32)
    for c in range(4):
        # whsa[(c,x), y, k] = wh[k, c, y, x]
        whsa[c * 32 : (c + 1) * 32] = wh[:, c].transpose(2, 1, 0)  # (x, y, k)
    whsb[:] = wh[:, 4].transpose(2, 1, 0)
    whsa = whsa.reshape(128, 320)
    whsb = whsb.reshape(32, 320)

    biasm = np.zeros((128, 8), np.float32)
    biasm[0:96, 0] = np.repeat(b1[0:3], 32)
    biasm[0:96, 1] = np.repeat(b1[3:6], 32)
    biasm[0:128, 2] = 0.5 * np.repeat(b2[0:4], 32)
    biasm[0:32, 3] = 0.5 * np.repeat(b2[4:5], 32)
    biasm[0:10, 4] = bh

    shared = {"w1s": w1s.astype(np.float16), "w2s": w2s.astype(np.float16), "whsa": whsa, "whsb": whsb, "bias": biasm}

    imgs = []
    for c in range(NCORES):
        sh = image[c * NPER : (c + 1) * NPER]  # [64, 3, 32, 32]
        slab = np.zeros((3, 32, Y, NPER), np.float32)  # (c, x, ypad, n)
        slab[:, :, 2:34, :] = sh.transpose(1, 3, 2, 0)
        slab = slab.reshape(96, Y, NPER)
        quads = [slab[:, 8 * q : 8 * q + 12, :].reshape(96, 12 * NPER) for q in range(4)]
        imgs.append(np.concatenate(quads, axis=1).astype(np.float16))
    return shared, imgs


_NC_CACHE = {}


def _get_nc(iters, unroll=5):
    key = (iters, unroll)
    if key not in _NC_CACHE:
        _NC_CACHE[key] = build_nc(iters, unroll)
    return _NC_CACHE[key]


def kernel(image, w1, b1, w2, b2, wh, bh, _iters=ITERS, _unroll=5):
    from concourse.bass_utils import run_bass_kernel_spmd

    shared, imgs = pack_inputs(image, w1, b1, w2, b2, wh, bh)
    in_maps = [dict(shared, img=imgs[c]) for c in range(NCORES)]
    nc = _get_nc(_iters, _unroll)
    res = run_bass_kernel_spmd(nc, in_maps, list(range(NCORES)))
    outs = []
    for c in range(NCORES):
        o = res.results[c]["out"]  # [10, 64]
        outs.append(o.T)  # [64, 10]
    logits = np.concatenate(outs, axis=0).astype(np.float32)  # [512, 10]
    return logits.reshape(NTOT, 10, 1, 1)

